# revision 28
# baseline (speedup 1.0000x reference)
"""v2-fixed reconstruction: bf16 spine kernel, per-rep prologue, rec parity fix."""

import numpy as np

B, N, D = 2, 2048, 1024
H, DH = 16, 64
SCALE = DH**-0.5
NCORES = 8
HLOC = H // 4
DLOC = HLOC * DH
P = 128
IB = 512
NIB = N // IB

MM_MODE = "bf16"

_cached = {}


def _build(mm_mode=MM_MODE, repeat=1):
    import concourse.bass as bass
    import concourse.tile as tile
    from concourse import bacc, mybir

    mm_mode, *variants = mm_mode.split("+")
    variants = set(variants)

    f32 = mybir.dt.float32
    f32r = mybir.dt.float32r

    if mm_mode == "bf16":
        io_dt = mybir.dt.bfloat16
    elif mm_mode == "f32r":
        io_dt = f32r
    else:
        io_dt = f32

    nc = bacc.Bacc("TRN2", target_bir_lowering=False, debug=False)

    xT = nc.dram_tensor("xT", [D, N], io_dt, kind="ExternalInput").ap()
    wqT = nc.dram_tensor("wqT", [D, DLOC], io_dt, kind="ExternalInput").ap()
    wkT = nc.dram_tensor("wkT", [D, DLOC], io_dt, kind="ExternalInput").ap()
    wvT = nc.dram_tensor("wvT", [D, DLOC], io_dt, kind="ExternalInput").ap()
    woutT = nc.dram_tensor("woutT", [DLOC, D], io_dt, kind="ExternalInput").ap()
    out = nc.dram_tensor("out", [N, D], f32, kind="ExternalOutput").ap()

    CT = D // P
    NT = N // P
    DT2 = DLOC // P

    with tile.TileContext(nc) as tc:
        with (
            tc.tile_pool(name="big", bufs=1) as big,
            tc.tile_pool(name="stage", bufs=3) as stage,
            tc.tile_pool(name="obst", bufs=2) as obst,
            tc.tile_pool(name="small", bufs=1) as small,
            tc.tile_pool(name="ps_s", bufs=2, space="PSUM") as ps_s,
            tc.tile_pool(name="ps_o", bufs=2, space="PSUM") as ps_o,
            tc.tile_pool(name="ps_op", bufs=2, space="PSUM") as ps_op,
        ):
            ones_sb = small.tile([1, DH], f32r, tag="ones")
            nc.vector._memset_packed(
                ones_sb[:].bitcast(mybir.dt.uint32), 0x3F800000
            )

            for rep in range(repeat):
                _emit_iter(
                    nc, tile, mybir, f32, f32r, io_dt, rep, variants,
                    big, stage, obst, small, ps_s, ps_o, ps_op, ones_sb,
                    xT, wqT, wkT, wvT, woutT, out,
                    CT, NT, DT2,
                )

    nc.compile()
    return nc


def _emit_iter(
    nc, tile, mybir, f32, f32r, io_dt, rep, variants,
    big, stage, obst, small, ps_s, ps_o, ps_op, ones_sb,
    xT, wqT, wkT, wvT, woutT, out,
    CT, NT, DT2,
):
    Exp = mybir.ActivationFunctionType.Exp

    xT_sb = big.tile([P, CT, N], io_dt, tag="xT", name=f"xT_sb_{rep}")
    wqT_sb = big.tile([P, CT, DLOC], io_dt, tag="wqT", name=f"wqT_sb_{rep}")
    wkT_sb = big.tile([P, CT, DLOC], io_dt, tag="wkT", name=f"wkT_sb_{rep}")
    wvT_sb = big.tile([P, CT, DLOC], io_dt, tag="wvT", name=f"wvT_sb_{rep}")
    woutT_sb = big.tile([P, DT2, D], io_dt, tag="woutT", name=f"woutT_sb_{rep}")
    qT_sb = big.tile([P, DT2, N], io_dt, tag="qT", name=f"qT_sb_{rep}")
    kT_sb = big.tile([P, DT2, N], io_dt, tag="kT", name=f"kT_sb_{rep}")
    v_sb = big.tile([P, NT, HLOC, DH + 1], io_dt, tag="v", name=f"v_sb_{rep}")
    oT_sb = big.tile([P, DT2, N], io_dt, tag="oT", name=f"oT_sb_{rep}")
    rec_sb = small.tile([1, 2, HLOC, IB], f32r, tag="rec", name=f"rec_{rep}")

    col = v_sb[:, :, :, DH]
    if io_dt == mybir.dt.bfloat16:
        nc.vector._memset_packed(col.bitcast(mybir.dt.uint16), 0x3F80)
    elif io_dt == mybir.dt.float32r:
        nc.vector._memset_packed(col.bitcast(mybir.dt.uint32), 0x3F800000)
    else:
        nc.vector.memset(col, 1.0)

    nc.sync.dma_start(wkT_sb[:], wkT.rearrange("(c p) d -> p c d", p=P))
    for ct in range(3):
        nc.sync.dma_start(xT_sb[:, ct, :], xT[ct * P : ct * P + P, :])
    nc.sync.dma_start(wqT_sb[:], wqT.rearrange("(c p) d -> p c d", p=P))
    for ct in range(3, CT):
        nc.sync.dma_start(xT_sb[:, ct, :], xT[ct * P : ct * P + P, :])
    nc.sync.dma_start(wvT_sb[:], wvT.rearrange("(c p) d -> p c d", p=P))
    nc.sync.dma_start(woutT_sb[:], woutT.rearrange("(t p) d -> p t d", p=P))

    def proj_chunk_mms(w_sb, dt_, i0, iw, ps):
        for ct in range(CT):
            for h0 in range(0, iw, 512):
                hw_ = min(512, iw - h0)
                yield lambda ct=ct, h0=h0, hw_=hw_: nc.tensor.matmul(
                    ps[:, h0 : h0 + hw_],
                    w_sb[:, ct, dt_ * P : dt_ * P + P],
                    xT_sb[:, ct, i0 + h0 : i0 + h0 + hw_],
                    start=(ct == 0),
                    stop=(ct == CT - 1),
                )

    def proj_emit(w_sb, dst, dt_, i0, iw, key):
        ps = ps_s.tile([P, 1024], f32, tag="s", name=f"ps_{key}_{rep}")
        for mm in proj_chunk_mms(w_sb, dt_, i0, iw, ps):
            mm()
        nc.vector.tensor_copy(dst[:, dt_, i0 : i0 + iw], ps[:, :iw])

    # k projection for keys j 0-1023 only; j 1024-2047 is deferred into
    # the spine filler queue (its chunks drain before the jt8 s-matmuls:
    # pops run at slot start, 2 per slot while backlogged, so the dt0
    # chunks' 16 matmuls complete within the first 8 slots).
    for dt_ in range(DT2):
        proj_emit(wkT_sb, kT_sb, dt_, 0, 1024, f"k0{dt_}")

    for jt in range(NT):
        psv = ps_s.tile([P, 1024], f32, tag="s", name=f"psv_{rep}_{jt}")
        for ct in range(CT):
            nc.tensor.matmul(
                psv[:, :DLOC],
                xT_sb[:, ct, jt * P : jt * P + P],
                wvT_sb[:, ct, :],
                start=(ct == 0),
                stop=(ct == CT - 1),
            )
        nc.vector.tensor_copy(
            v_sb[:, jt, :, :DH],
            psv[:, :DLOC].rearrange("p (h u) -> p h u", u=DH),
        )

    for dt_ in range(DT2):
        proj_emit(wqT_sb, qT_sb, dt_, 0, IB, f"q0{dt_}")

    fillers = []

    def push_proj_filler(w_sb, dst, dt_, i0, key):
        ps_box = {}

        for idx in range(CT):
            def step(idx=idx):
                if idx == 0:
                    ps_box["ps"] = ps_op.tile(
                        [P, 512], f32, tag="op", name=f"ps_{key}_{rep}"
                    )
                nc.tensor.matmul(
                    ps_box["ps"][:],
                    w_sb[:, idx, dt_ * P : dt_ * P + P],
                    xT_sb[:, idx, i0 : i0 + IB],
                    start=(idx == 0),
                    stop=(idx == CT - 1),
                )
            fillers.append((1, step))
        fillers.append(
            (0, lambda: nc.vector.tensor_copy(
                dst[:, dt_, i0 : i0 + IB], ps_box["ps"][:]))
        )

    for dt_ in range(DT2):
        for i0 in (1024, 1536):
            push_proj_filler(wkT_sb, kT_sb, dt_, i0, f"k1{dt_}{i0}")
    for ib in range(1, NIB):
        for dt_ in range(DT2):
            push_proj_filler(wqT_sb, qT_sb, dt_, ib * IB, f"q{ib}{dt_}")

    def push_norm_filler(ib, h):
        hp, ho = h // 2, (h % 2) * DH
        i0 = ib * IB
        box = {}

        def bc_mm():
            bc = ps_op.tile([P, 512], f32, tag="op", name=f"bc_{rep}_{ib}_{h}")
            box["bc"] = bc
            nc.tensor.matmul(
                bc[:DH, :IB],
                ones_sb[:],
                rec_sb[:, ib % 2, h, :],
                start=True,
                stop=True,
            )

        def mul():
            dst = oT_sb[ho : ho + DH, hp, i0 : i0 + IB]
            nc.vector.tensor_mul(dst, dst, box["bc"][:DH, :IB])

        fillers.append((1, bc_mm))
        fillers.append((0, mul))

    def push_outproj_filler(ib):
        def push_one(it):
            ob_box = {}

            def alloc_ob():
                ob_box["ob"] = obst.tile(
                    [P, 1024], f32, tag="ob", name=f"ob_{rep}_{it}"
                )

            fillers.append((0, alloc_ob))

            def push_half(db):
                pp_box = {}

                def mk_mm(dt_):
                    def mm():
                        if dt_ == 0:
                            pp_box["pp"] = ps_op.tile(
                                [P, 512], f32, tag="op",
                                name=f"pso_{rep}_{it}_{db}",
                            )
                        nc.tensor.matmul(
                            pp_box["pp"][:],
                            oT_sb[:, dt_, it * P : it * P + P],
                            woutT_sb[:, dt_, db * 512 : db * 512 + 512],
                            start=(dt_ == 0),
                            stop=(dt_ == DT2 - 1),
                        )
                    return mm

                for dt_ in range(DT2):
                    fillers.append((1, mk_mm(dt_)))
                fillers.append(
                    (0, lambda: nc.vector.tensor_copy(
                        ob_box["ob"][:, db * 512 : db * 512 + 512],
                        pp_box["pp"][:]))
                )

            for db in range(2):
                push_half(db)
            fillers.append(
                (0, lambda: nc.sync.dma_start(
                    out[it * P : it * P + P, :], ob_box["ob"][:]))
            )

        for k in range(4):
            push_one(ib * 4 + k)

    def pop_fillers(budget=None):
        if budget is None:
            budget = 2 if len(fillers) > 16 else 1
        spent = 0
        while fillers and spent < budget:
            cost, fn = fillers.pop(0)
            fn()
            spent += cost
        while fillers and fillers[0][0] == 0:
            fillers.pop(0)[1]()

    for ib in range(NIB):
        i0 = ib * IB
        for hp in range(DT2):
            hA, hB = 2 * hp, 2 * hp + 1
            poA = ps_o.tile([DH + 1, IB], f32, tag="o", name=f"po_{rep}_{ib}_{hA}")
            poB = ps_o.tile([DH + 1, IB], f32, tag="o", name=f"po_{rep}_{ib}_{hB}")
            for jt in range(NT):
                pop_fillers()
                pss = ps_s.tile(
                    [P, 1024], f32, tag="s", name=f"pss_{rep}_{ib}_{hp}_{jt}"
                )
                nc.tensor.matmul(
                    pss[:, 0:512],
                    kT_sb[0:DH, hp, jt * P : jt * P + P],
                    qT_sb[0:DH, hp, i0 : i0 + IB],
                    start=True, stop=True,
                )
                nc.tensor.matmul(
                    pss[:, 512:1024],
                    kT_sb[DH:P, hp, jt * P : jt * P + P],
                    qT_sb[DH:P, hp, i0 : i0 + IB],
                    start=True, stop=True,
                )
                p_sb = stage.tile(
                    [P, 1024], io_dt, tag="p", name=f"p_{rep}_{ib}_{hp}_{jt}"
                )
                nc.scalar.activation(p_sb[:], pss[:], Exp, scale=SCALE)
                nc.tensor.matmul(
                    poA[:],
                    v_sb[:, jt, hA, :],
                    p_sb[:, 0:512],
                    start=(jt == 0), stop=(jt == NT - 1),
                )
                nc.tensor.matmul(
                    poB[:],
                    v_sb[:, jt, hB, :],
                    p_sb[:, 512:1024],
                    start=(jt == 0), stop=(jt == NT - 1),
                )
            for h, po in ((hA, poA), (hB, poB)):
                ho = (h % 2) * DH
                with nc.allow_low_precision(reason="f32r recip is full fp32"):
                    nc.vector.reciprocal(
                        rec_sb[:, ib % 2, h, :], po[DH : DH + 1, :]
                    )
                nc.vector.tensor_copy(
                    oT_sb[ho : ho + DH, hp, i0 : i0 + IB], po[:DH, :]
                )
                push_norm_filler(ib, h)
        push_outproj_filler(ib)

    while fillers:
        fillers.pop(0)[1]()


def get_nc(mm_mode=MM_MODE, repeat=1):
    key = (mm_mode, repeat)
    if key not in _cached:
        _cached[key] = _build(mm_mode, repeat)
    return _cached[key]


def make_in_maps(x, Wq, Wk, Wv, Wout, mm_mode=MM_MODE):
    mm_mode = mm_mode.split("+")[0]
    if mm_mode == "bf16":
        import ml_dtypes

        cast = lambda a: np.ascontiguousarray(np.asarray(a), dtype=ml_dtypes.bfloat16)
    else:
        cast = lambda a: np.ascontiguousarray(np.asarray(a), dtype=np.float32)
    x, Wq, Wk, Wv, Wout = (np.asarray(a) for a in (x, Wq, Wk, Wv, Wout))
    in_maps = []
    for c in range(NCORES):
        b = c // 4
        rows = slice((c % 4) * DLOC, (c % 4 + 1) * DLOC)
        in_maps.append(
            {
                "xT": cast(x[b].T),
                "wqT": cast(Wq[rows].T),
                "wkT": cast(Wk[rows].T),
                "wvT": cast(Wv[rows].T),
                "woutT": cast(Wout[:, rows].T),
            }
        )
    return in_maps


def kernel(x, Wq, Wk, Wv, Wout, bout):
    from concourse.bass_utils import run_bass_kernel_spmd

    nc = get_nc()
    in_maps = make_in_maps(x, Wq, Wk, Wv, Wout)
    res = run_bass_kernel_spmd(nc, in_maps, list(range(NCORES)))
    out = np.zeros((B, N, D), np.float32)
    for c in range(NCORES):
        out[c // 4] += res.results[c]["out"]
    out += np.asarray(bout, np.float32)
    return out
